# revision 5
# baseline (speedup 1.0000x reference)
"""Causal multi-head attention on 8 TRN2 NeuronCores — fused pipeline v2.

Problem: x[4,2048,1024], w_attn[1024,3072], w_proj[1024,1024],
16 heads x 64 dim, causal softmax(QK^T/8)V then output projection.

Sharding: 4-way batch x 2-way head-half. Each core computes a partial
y^T; the host sums the two partials per batch and transposes.

Design, driven by HW microbenchmarks (see microbench.py):
 - ACT instructions cost ~+330ns over the cost model each, so ACT does
   ONLY the exp (the one op no other engine can do); every PSUM
   evacuation moved to DVE (measured DVE PSUM reads are ~1.3-1.5x
   model, NOT the 5x the previous kernel assumed).
 - exp count minimized: S steps cover TWO key tiles x two head
   parities in one [128,4,512] PSUM tile -> one 2048-wide exp for
   non-diagonal steps (112 exps instead of 160).
 - S^T matmuls are K=64 row-tiled pairs at base partitions 0/64
   (tile_position (0,0)/(64,0)) - microbenched ~2x concurrent on HW.
 - Fused schedule: no stages. Work is emitted qc-major, one unit
   ahead: Q/K chunk projections (and V tiles) of unit u+1 are slotted
   between the S/exp steps of unit u and PV chunks of unit u-1, so PE
   (projection+PV-heavy) and ACT (exp-heavy) both stay fed. C m-tile
   pairs drain lazily after each qc's last pair retires.
 - Q and K of a pair live in ONE [128, 2, tpc] SBUF tile and their
   projections in one [128,2,512] PSUM tile -> single DVE evac per
   unit; V evacs in kt-pairs into one vhat tensor; C in m-pairs.
 - PV with stationary [V_h | 1] (65 cols): PSUM row 64 accumulates the
   softmax denominators for free; po holds both parities [128,2,512]
   so ONE evac moves O^T+den; norm = 4 den row copies into bci rows
   {0,32,64,96}, one stream_shuffle, one reciprocal, two bf16 muls.
 - Diagonal masks + ones fills on GPSIMD (Pool), the idle engine.
 - PSUM: S 4 banks + projections/C 2 + po 2 = 8 exactly.
"""

import numpy as np
from contextlib import ExitStack

import concourse.bass as bass
import concourse.tile as tile
from concourse import bacc, mybir
from concourse.bass_utils import run_bass_kernel_spmd

f32 = mybir.dt.float32
bf16 = mybir.dt.bfloat16
EXP = mybir.ActivationFunctionType.Exp
COPY = mybir.ActivationFunctionType.Copy

B, T, C = 4, 2048, 1024
N_HEAD, HD = 16, 64
HPC = 8            # heads per core
FS = HPC * HD      # 512: per-core feature slice for each of q/k/v
NPAIR = HPC // 2   # 4 head pairs
SCALE = 1.0 / 8.0  # 1/sqrt(64)
N_CORES = 8


def build_nc(tpc=T, loop_n=1, dyn_loop=0, ob_eng="dve", bci_eng="dve",
             mask_eng="pool", pp_bufs=15):
    """Build the single-core Bass program (SPMD: same program all cores)."""
    nck = C // 128          # 8 c_in tiles
    nkt = tpc // 128        # key tiles
    nqc = tpc // 512        # query chunks (512 wide)
    nmt = C // 128          # 8 output-channel tiles

    nc = bacc.Bacc("TRN2", target_bir_lowering=False)
    xt = nc.dram_tensor("xt", [C, tpc], bf16, kind="ExternalInput")
    wq = nc.dram_tensor("wq", [C, FS], bf16, kind="ExternalInput")
    wk = nc.dram_tensor("wk", [C, FS], bf16, kind="ExternalInput")
    wv = nc.dram_tensor("wv", [C, FS], bf16, kind="ExternalInput")
    wp = nc.dram_tensor("wp", [FS, C], bf16, kind="ExternalInput")
    mk = nc.dram_tensor("mk", [128, 128], bf16, kind="ExternalInput")
    yt = nc.dram_tensor("yt", [C, tpc], f32, kind="ExternalOutput")

    with tile.TileContext(nc) as tc, ExitStack() as _dl:
     if dyn_loop:
        _dl.enter_context(tc.For_i(0, dyn_loop, 1))
     for _rep in range(loop_n):
      with ExitStack() as stk:
        persist = stk.enter_context(tc.tile_pool(name="persist", bufs=1))
        # q/k of a pair share one tile: [:, 0, :] = q^T, [:, 1, :] = k^T
        qkT = [persist.tile([128, 2, tpc], bf16, tag=f"qk{p}", name=f"qk{p}")
               for p in range(NPAIR)]
        vhat = persist.tile([128, nkt, HPC, 65], bf16, tag="vh", name="vh")
        oT = [persist.tile([128, tpc], bf16, tag=f"oT{p}", name=f"oT{p}")
              for p in range(NPAIR)]
        mkt = persist.tile([128, 128], bf16, tag="mk")
        ones_f = persist.tile([128, HPC], f32, tag="ones")
        nc.vector.memset(ones_f[:, :], 1.0)
        bcis = []
        for bi in range(2):
            b_ = persist.tile([64, 512], bf16, tag=f"bci{bi}", name=f"bci{bi}")
            nc.vector.memset(b_[:, :], 1.0)
            bcis.append(b_)

        # ---- inputs: fine-grained DMAs ordered by first use ----
        xts = [persist.tile([128, tpc], bf16, tag=f"x{i}", name=f"x{i}")
               for i in range(nck)]
        wqt = persist.tile([128, nck, FS], bf16, tag="wq", name="wqt")
        wkt = persist.tile([128, nck, FS], bf16, tag="wk", name="wkt")
        wvt = persist.tile([128, nck, FS], bf16, tag="wv", name="wvt")
        wpt = persist.tile([128, NPAIR, C], bf16, tag="wp", name="wpt")
        xsrc = xt.rearrange("(a p) f -> p a f", p=128)
        wqsrc = wq.rearrange("(a p) f -> p a f", p=128)
        wksrc = wk.rearrange("(a p) f -> p a f", p=128)
        wvsrc = wv.rearrange("(a p) f -> p a f", p=128)
        wpsrc = wp.rearrange("(a p) f -> p a f", p=128)
        for k in range(nck):   # interleaved so matmul k can start early
            nc.sync.dma_start(out=wqt[:, k, :], in_=wqsrc[:, k, :])
            nc.sync.dma_start(out=wkt[:, k, :], in_=wksrc[:, k, :])
            nc.sync.dma_start(out=xts[k][:, 0:512], in_=xsrc[:, k, 0:512])
        nc.sync.dma_start(out=mkt, in_=mk[:, :])
        for k in range(nck):
            nc.sync.dma_start(out=wvt[:, k, :], in_=wvsrc[:, k, :])
        for qc in range(1, nqc):
            for k in range(nck):
                nc.sync.dma_start(out=xts[k][:, qc * 512:(qc + 1) * 512],
                                  in_=xsrc[:, k, qc * 512:(qc + 1) * 512])
        for j in range(NPAIR):
            nc.sync.dma_start(out=wpt[:, j, :], in_=wpsrc[:, j, :])

        pp = stk.enter_context(tc.tile_pool(name="pp", bufs=pp_bufs))
        rp = stk.enter_context(tc.tile_pool(name="rp", bufs=2))
        ev = stk.enter_context(tc.tile_pool(name="ev", bufs=2))
        psA = stk.enter_context(tc.tile_pool(name="psA", bufs=1, space="PSUM"))
        psS = stk.enter_context(tc.tile_pool(name="psS", bufs=1, space="PSUM"))
        psO = stk.enter_context(tc.tile_pool(name="psO", bufs=1, space="PSUM"))

        mask_ns = nc.gpsimd if mask_eng == "pool" else nc.vector
        bci_ns = nc.vector if bci_eng == "dve" else nc.gpsimd

        units = [(p, qc) for qc in range(nqc) for p in range(NPAIR)]
        pend_c = []

        # ---------------- emitters ----------------
        def task_qk(p, qc):
            """Project q & k chunk qc of pair p into one 2-bank PSUM tile."""
            qsl = slice(qc * 512, (qc + 1) * 512)
            ps = psA.tile([128, 2, 512], f32, tag="pa", name="pa")
            for which, wt in ((0, wqt), (1, wkt)):
                for k in range(nck):
                    nc.tensor.matmul(
                        ps[:, which, :], wt[:, k, p * 128:(p + 1) * 128],
                        xts[k][:, qsl], start=(k == 0), stop=(k == nck - 1))
            nc.vector.tensor_copy(qkT[p][:, :, qsl], ps[:, :, :])

        def task_v(kt0):
            """Project V for key tiles kt0, kt0+1 into one 2-bank tile."""
            ps = psA.tile([128, 2, 512], f32, tag="pa", name="pa")
            for i, kt in ((0, kt0), (1, kt0 + 1)):
                for k in range(nck):
                    nc.tensor.matmul(
                        ps[:, i, :], xts[k][:, kt * 128:(kt + 1) * 128],
                        wvt[:, k, :], start=(k == 0), stop=(k == nck - 1))
            nc.vector.tensor_copy(
                vhat[:, kt0:kt0 + 2, :, 0:HD],
                ps.rearrange("p t (h d) -> p t h d", h=HPC))
            nc.gpsimd.tensor_copy(
                vhat[:, kt0:kt0 + 2, :, HD],
                ones_f[:, None, 0:HPC].broadcast_to([128, 2, HPC]))

        def a_tasks(idx):
            """A-work closures for unit idx (projections one unit ahead)."""
            if idx >= len(units):
                return []
            p, qc = units[idx]
            ts = [lambda p=p, qc=qc: task_qk(p, qc)]
            if p == 0:
                ts.append(lambda kt0=4 * qc: task_v(kt0))
                ts.append(lambda kt0=4 * qc + 2: task_v(kt0))
            return ts

        def emit_s_step(st):
            """One S^T step: 2 key tiles x 2 parities in one [128,4,512]
            PSUM tile; one 2048-wide exp (non-diag) or two per-kt exps +
            masks (diag). Returns False when done."""
            p, qc, steps, si = st["p"], st["qc"], st["steps"], st["si"]
            if si >= len(steps):
                return False
            typ, ka, kb = steps[si]
            ps = psS.tile([128, 4, 512], f32, tag="s", name="s")
            pr = pp.tile([128, 4, 512], bf16, tag="P", name="P")
            # diag steps: both kts' matmuls span [off_a:512] so one exp can
            # drain the whole tile; kt_b's [off_a:off_b) columns are computed
            # but never read (PV uses the true off from ptiles)
            off_mm = 128 * (ka % 4) if typ == "dg" else 0
            for sub, kt in ((0, ka), (1, kb)):
                off = 128 * (kt % 4) if typ == "dg" else 0
                ksl = slice(kt * 128, (kt + 1) * 128)
                qs2 = slice(qc * 512 + off_mm, (qc + 1) * 512)
                for par in range(2):   # row-tiled pair (base partitions 0/64)
                    row = slice(64 * par, 64 * par + 64)
                    nc.tensor.matmul(
                        ps[:, 2 * sub + par, off_mm:512], qkT[p][row, 1, ksl],
                        qkT[p][row, 0, qs2], start=True, stop=True)
                st["ptiles"][kt] = (pr, sub, off)
            if typ == "nd":
                nc.scalar.activation(pr[:, :, :], ps[:, :, :], EXP,
                                     scale=SCALE)
            else:
                # one exp over [off_a:512] of all 4 banks
                nc.scalar.activation(pr[:, :, off_mm:512],
                                     ps[:, :, off_mm:512], EXP, scale=SCALE)
                for sub, kt in ((0, ka), (1, kb)):
                    off = 128 * (kt % 4)
                    mask_ns.tensor_mul(
                        pr[:, 2 * sub:2 * sub + 2, off:off + 128],
                        pr[:, 2 * sub:2 * sub + 2, off:off + 128],
                        mkt[:, None, :].broadcast_to([128, 2, 128]))
            st["si"] += 1
            return True

        def emit_pv_chunk(st, n=2):
            """Up to n PV key tiles per parity as same-bank matmul runs."""
            kts, j0 = st["kts"], st["j"]
            if j0 >= len(kts):
                return False
            if st["po"] is None:
                st["po"] = psO.tile([128, 2, 512], f32, tag="po", name="po")
            po = st["po"]
            j1 = min(j0 + n, len(kts))
            p = st["p"]
            for par in range(2):
                for j in range(j0, j1):
                    kt = kts[j]
                    pr, sub, off = st["ptiles"][kt]
                    nc.tensor.matmul(
                        po[0:65, par, off:512],
                        vhat[:, kt, 2 * p + par, :],
                        pr[:, 2 * sub + par, off:512],
                        start=(kt == 0), stop=(kt == kts[-1]))
            st["j"] = j1
            return True

        def emit_norm(st):
            """oT = po[0:64] * 1/po[64], both parities batched."""
            p, qc = st["p"], st["qc"]
            qsl = slice(qc * 512, (qc + 1) * 512)
            po = st["po"]
            ob = rp.tile([65, 2, 512], bf16, tag="ob", name="ob")
            if ob_eng == "act":
                nc.scalar.activation(ob[:, :, :], po[0:65, :, :], COPY)
            else:
                nc.vector.tensor_copy(ob[:, :, :], po[0:65, :, :])
            for par in range(2):
                bci = bcis[par]
                bci_ns.tensor_copy(bci[0:1, :], ob[64:65, par, :])
                bci_ns.tensor_copy(bci[32:33, :], ob[64:65, par, :])
                bc = rp.tile([64, 512], bf16, tag="bc", name="bc")
                nc.vector.stream_shuffle(bc[:, :], bci[:, :], [0] * 32)
                rden = rp.tile([64, 512], bf16, tag="rden", name="rden")
                with nc.allow_low_precision(
                        reason="bf16 softmax denominators"):
                    nc.vector.reciprocal(rden[:, :], bc[:, :])
                nc.vector.tensor_mul(oT[p][64 * par:64 * par + 64, qsl],
                                     ob[0:64, par, :], rden[:, :])

        def emit_c_mpair(qc, m0):
            """Output-projection m-tiles m0, m0+1 in one 2-bank PSUM tile."""
            ps = psA.tile([128, 2, 512], f32, tag="pa", name="pa")
            for i, m in ((0, m0), (1, m0 + 1)):
                for j in range(NPAIR):
                    nc.tensor.matmul(
                        ps[:, i, :], wpt[:, j, m * 128:(m + 1) * 128],
                        oT[j][:, qc * 512:(qc + 1) * 512],
                        start=(j == 0), stop=(j == NPAIR - 1))
            sb = ev.tile([128, 2, 512], f32, tag="sb", name="sb")
            nc.vector.tensor_copy(sb[:, :, :], ps[:, :, :])
            nc.sync.dma_start(
                out=yt[m0 * 128:(m0 + 2) * 128,
                       qc * 512:(qc + 1) * 512].rearrange(
                           "(t p) f -> p t f", p=128),
                in_=sb)

        def new_state(p, qc):
            kts = list(range(min(nkt, 4 * (qc + 1))))
            nd = 4 * qc
            steps = ([("nd", kts[i], kts[i + 1]) for i in range(0, nd, 2)]
                     + [("dg", kts[nd], kts[nd + 1]),
                        ("dg", kts[nd + 2], kts[nd + 3])])
            return {"p": p, "qc": qc, "kts": kts, "steps": steps, "si": 0,
                    "j": 0, "ptiles": {}, "po": None}

        def retire(st):
            while emit_pv_chunk(st, n=4):
                pass
            emit_norm(st)
            if st["p"] == NPAIR - 1:
                pend_c.extend((st["qc"], m0) for m0 in range(0, nmt, 2))

        def drain_c(k):
            for _ in range(min(k, len(pend_c))):
                emit_c_mpair(*pend_c.pop(0))

        # ---------------- main pipeline ----------------
        # Filler discipline: PE is in-order, and each S step's matmuls
        # must wait for the previous step's exp (single-buffered [128,4,512]
        # S tile). So between S-mm batches PE needs ~2.5us of other queued
        # work: a PV chunk (n=4) plus one filler (V projection or C m-pair).
        for t in a_tasks(0):
            t()
        prev = None
        for idx, (p, qc) in enumerate(units):
            atasks = a_tasks(idx + 1)
            if atasks:
                atasks.pop(0)()          # q/k projection of next unit
            filler = atasks              # remaining V tasks, then C drains
            cur = new_state(p, qc)
            more_s = True
            while more_s:
                more_s = emit_s_step(cur)
                if prev is not None:
                    emit_pv_chunk(prev, n=4)
                    if prev["j"] >= len(prev["kts"]):
                        retire(prev)
                        prev = None
                if filler:
                    filler.pop(0)()
                elif pend_c and more_s:
                    emit_c_mpair(*pend_c.pop(0))
            for t in filler:
                t()
            if prev is not None:
                retire(prev)
            prev = cur
        if prev is not None:
            retire(prev)
        drain_c(len(pend_c))
    nc.compile()
    return nc


def _make_masks():
    import ml_dtypes
    k = np.arange(128)[:, None]
    q = np.arange(128)[None, :]
    return (q >= k).astype(ml_dtypes.bfloat16)


_NC_CACHE = {}


def _get_nc(tpc=T):
    if tpc not in _NC_CACHE:
        _NC_CACHE[tpc] = build_nc(tpc)
    return _NC_CACHE[tpc]


def make_in_maps(x, w_attn, w_proj):
    import ml_dtypes
    bf = ml_dtypes.bfloat16
    masks = _make_masks()
    in_maps = []
    for core in range(N_CORES):
        b, hh = core // 2, core % 2
        s = slice(hh * FS, (hh + 1) * FS)
        in_maps.append({
            "xt": np.ascontiguousarray(np.asarray(x[b]).T).astype(bf),
            "wq": np.ascontiguousarray(w_attn[:, s]).astype(bf),
            "wk": np.ascontiguousarray(w_attn[:, C:][:, s]).astype(bf),
            "wv": np.ascontiguousarray(w_attn[:, 2 * C:][:, s]).astype(bf),
            "wp": np.ascontiguousarray(w_proj[hh * FS:(hh + 1) * FS, :]).astype(bf),
            "mk": masks,
        })
    return in_maps


def kernel(x, w_attn, w_proj):
    nc = _get_nc(T)
    in_maps = make_in_maps(x, w_attn, w_proj)
    res = run_bass_kernel_spmd(nc, in_maps, list(range(N_CORES)))
    y = np.empty((B, T, C), np.float32)
    for b in range(B):
        yt = res.results[2 * b]["yt"] + res.results[2 * b + 1]["yt"]
        y[b] = yt.T
    return y


# revision 6
# speedup vs baseline: 1.1719x; 1.1719x over previous
"""Causal multi-head attention on 8 TRN2 NeuronCores — fused pipeline v2.

Problem: x[4,2048,1024], w_attn[1024,3072], w_proj[1024,1024],
16 heads x 64 dim, causal softmax(QK^T/8)V then output projection.

Sharding: 4-way batch x 2-way head-half. Each core computes a partial
y^T; the host sums the two partials per batch and transposes.

Design, driven by HW microbenchmarks (see microbench.py):
 - ACT instructions cost ~+330ns over the cost model each, so ACT does
   ONLY the exp (the one op no other engine can do); every PSUM
   evacuation moved to DVE (measured DVE PSUM reads are ~1.3-1.5x
   model, NOT the 5x the previous kernel assumed).
 - exp count minimized: S steps cover TWO key tiles x two head
   parities in one [128,4,512] PSUM tile -> one 2048-wide exp for
   non-diagonal steps (112 exps instead of 160).
 - S^T matmuls are K=64 row-tiled pairs at base partitions 0/64
   (tile_position (0,0)/(64,0)) - microbenched ~2x concurrent on HW.
 - Fused schedule: no stages. Work is emitted qc-major, one unit
   ahead: Q/K chunk projections (and V tiles) of unit u+1 are slotted
   between the S/exp steps of unit u and PV chunks of unit u-1, so PE
   (projection+PV-heavy) and ACT (exp-heavy) both stay fed. C m-tile
   pairs drain lazily after each qc's last pair retires.
 - Q and K of a pair live in ONE [128, 2, tpc] SBUF tile and their
   projections in one [128,2,512] PSUM tile -> single DVE evac per
   unit; V evacs in kt-pairs into one vhat tensor; C in m-pairs.
 - PV with stationary [V_h | 1] (65 cols): PSUM row 64 accumulates the
   softmax denominators for free; po holds both parities [128,2,512]
   so ONE evac moves O^T+den; norm = 4 den row copies into bci rows
   {0,32,64,96}, one stream_shuffle, one reciprocal, two bf16 muls.
 - Diagonal masks + ones fills on GPSIMD (Pool), the idle engine.
 - PSUM: S 4 banks + projections/C 2 + po 2 = 8 exactly.
"""

import numpy as np
from contextlib import ExitStack

import concourse.bass as bass
import concourse.tile as tile
from concourse import bacc, mybir
from concourse.bass_utils import run_bass_kernel_spmd

f32 = mybir.dt.float32
bf16 = mybir.dt.bfloat16
EXP = mybir.ActivationFunctionType.Exp
COPY = mybir.ActivationFunctionType.Copy

B, T, C = 4, 2048, 1024
N_HEAD, HD = 16, 64
HPC = 8            # heads per core
FS = HPC * HD      # 512: per-core feature slice for each of q/k/v
NPAIR = HPC // 2   # 4 head pairs
SCALE = 1.0 / 8.0  # 1/sqrt(64)
N_CORES = 8


def build_nc(tpc=T, loop_n=1, dyn_loop=0, ob_eng="dve", bci_eng="dve",
             mask_eng="pool", pp_bufs=26):
    """Build the single-core Bass program (SPMD: same program all cores)."""
    nck = C // 128          # 8 c_in tiles
    nkt = tpc // 128        # key tiles
    nqc = tpc // 512        # query chunks (512 wide)
    nmt = C // 128          # 8 output-channel tiles

    nc = bacc.Bacc("TRN2", target_bir_lowering=False)
    xt = nc.dram_tensor("xt", [C, tpc], bf16, kind="ExternalInput")
    wq = nc.dram_tensor("wq", [C, FS], bf16, kind="ExternalInput")
    wk = nc.dram_tensor("wk", [C, FS], bf16, kind="ExternalInput")
    wv = nc.dram_tensor("wv", [C, FS], bf16, kind="ExternalInput")
    wp = nc.dram_tensor("wp", [FS, C], bf16, kind="ExternalInput")
    mk = nc.dram_tensor("mk", [128, 128], bf16, kind="ExternalInput")
    yt = nc.dram_tensor("yt", [C, tpc], f32, kind="ExternalOutput")

    with tile.TileContext(nc) as tc, ExitStack() as _dl:
     if dyn_loop:
        _dl.enter_context(tc.For_i(0, dyn_loop, 1))
     for _rep in range(loop_n):
      with ExitStack() as stk:
        persist = stk.enter_context(tc.tile_pool(name="persist", bufs=1))
        # q/k of a pair share one tile: [:, 0, :] = q^T, [:, 1, :] = k^T
        qkT = [persist.tile([128, 2, tpc], bf16, tag=f"qk{p}", name=f"qk{p}")
               for p in range(NPAIR)]
        vhat = persist.tile([128, nkt, HPC, 65], bf16, tag="vh", name="vh")
        oT = [persist.tile([128, tpc], bf16, tag=f"oT{p}", name=f"oT{p}")
              for p in range(NPAIR)]
        mkt = persist.tile([128, 128], bf16, tag="mk")
        ones_f = persist.tile([128, HPC], f32, tag="ones")
        nc.vector.memset(ones_f[:, :], 1.0)
        bcis = []
        for bi in range(2):
            b_ = persist.tile([64, 512], bf16, tag=f"bci{bi}", name=f"bci{bi}")
            nc.vector.memset(b_[:, :], 1.0)
            bcis.append(b_)

        # ---- inputs: fine-grained DMAs ordered by first use ----
        xts = [persist.tile([128, tpc], bf16, tag=f"x{i}", name=f"x{i}")
               for i in range(nck)]
        wqt = persist.tile([128, nck, FS], bf16, tag="wq", name="wqt")
        wkt = persist.tile([128, nck, FS], bf16, tag="wk", name="wkt")
        wvt = persist.tile([128, nck, FS], bf16, tag="wv", name="wvt")
        wpt = persist.tile([128, NPAIR, C], bf16, tag="wp", name="wpt")
        xsrc = xt.rearrange("(a p) f -> p a f", p=128)
        wqsrc = wq.rearrange("(a p) f -> p a f", p=128)
        wksrc = wk.rearrange("(a p) f -> p a f", p=128)
        wvsrc = wv.rearrange("(a p) f -> p a f", p=128)
        wpsrc = wp.rearrange("(a p) f -> p a f", p=128)
        for k in range(nck):   # interleaved so matmul k can start early
            nc.sync.dma_start(out=wqt[:, k, :], in_=wqsrc[:, k, :])
            nc.sync.dma_start(out=wkt[:, k, :], in_=wksrc[:, k, :])
            nc.sync.dma_start(out=xts[k][:, 0:512], in_=xsrc[:, k, 0:512])
        nc.sync.dma_start(out=mkt, in_=mk[:, :])
        for k in range(nck):
            nc.sync.dma_start(out=wvt[:, k, :], in_=wvsrc[:, k, :])
        for qc in range(1, nqc):
            for k in range(nck):
                nc.sync.dma_start(out=xts[k][:, qc * 512:(qc + 1) * 512],
                                  in_=xsrc[:, k, qc * 512:(qc + 1) * 512])
        for j in range(NPAIR):
            nc.sync.dma_start(out=wpt[:, j, :], in_=wpsrc[:, j, :])

        pp = stk.enter_context(tc.tile_pool(name="pp", bufs=pp_bufs))
        rp = stk.enter_context(tc.tile_pool(name="rp", bufs=2))
        ev = stk.enter_context(tc.tile_pool(name="ev", bufs=2))
        # one shared 3-slot rotation of [128,2,512] tiles serves S steps,
        # Q/K + V projections and C m-pairs (6 banks), po gets the last 2
        ws = stk.enter_context(tc.tile_pool(name="ws", bufs=3, space="PSUM"))
        psO = stk.enter_context(tc.tile_pool(name="psO", bufs=1, space="PSUM"))

        mask_ns = nc.gpsimd if mask_eng == "pool" else nc.vector
        bci_ns = nc.vector if bci_eng == "dve" else nc.gpsimd

        units = [(p, qc) for qc in range(nqc) for p in range(NPAIR)]
        pend_c = []

        # ---------------- emitters ----------------
        def task_qk(p, qc):
            """Project q & k chunk qc of pair p into one 2-bank PSUM tile."""
            qsl = slice(qc * 512, (qc + 1) * 512)
            ps = ws.tile([128, 2, 512], f32, tag="w", name="w")
            for which, wt in ((0, wqt), (1, wkt)):
                for k in range(nck):
                    nc.tensor.matmul(
                        ps[:, which, :], wt[:, k, p * 128:(p + 1) * 128],
                        xts[k][:, qsl], start=(k == 0), stop=(k == nck - 1))
            nc.vector.tensor_copy(qkT[p][:, :, qsl], ps[:, :, :])

        def task_v(kt0):
            """Project V for key tiles kt0, kt0+1 into one 2-bank tile."""
            ps = ws.tile([128, 2, 512], f32, tag="w", name="w")
            for i, kt in ((0, kt0), (1, kt0 + 1)):
                for k in range(nck):
                    nc.tensor.matmul(
                        ps[:, i, :], xts[k][:, kt * 128:(kt + 1) * 128],
                        wvt[:, k, :], start=(k == 0), stop=(k == nck - 1))
            nc.vector.tensor_copy(
                vhat[:, kt0:kt0 + 2, :, 0:HD],
                ps.rearrange("p t (h d) -> p t h d", h=HPC))
            nc.gpsimd.tensor_copy(
                vhat[:, kt0:kt0 + 2, :, HD],
                ones_f[:, None, 0:HPC].broadcast_to([128, 2, HPC]))

        def a_tasks(idx):
            """A-work closures for unit idx (projections one unit ahead)."""
            if idx >= len(units):
                return []
            p, qc = units[idx]
            ts = [lambda p=p, qc=qc: task_qk(p, qc)]
            if p == 0:
                ts.append(lambda kt0=4 * qc: task_v(kt0))
                ts.append(lambda kt0=4 * qc + 2: task_v(kt0))
            return ts

        def emit_s_step(st):
            """One S^T step: one key tile x 2 parities in a [128,2,512]
            PSUM tile from the shared 3-slot rotation; one 1024-wide exp
            (less for diagonal tiles). Returns False when done."""
            p, qc, kts, i = st["p"], st["qc"], st["kts"], st["i"]
            if i >= len(kts):
                return False
            kt = kts[i]
            diag = (kt // 4 == qc)
            off = 128 * (kt % 4) if diag else 0
            ksl = slice(kt * 128, (kt + 1) * 128)
            qs2 = slice(qc * 512 + off, (qc + 1) * 512)
            ps = ws.tile([128, 2, 512], f32, tag="w", name="w")
            for par in range(2):   # row-tiled pair (base partitions 0/64)
                row = slice(64 * par, 64 * par + 64)
                nc.tensor.matmul(
                    ps[:, par, off:512], qkT[p][row, 1, ksl],
                    qkT[p][row, 0, qs2], start=True, stop=True)
            pr = pp.tile([128, 2, 512], bf16, tag="P", name="P")
            nc.scalar.activation(pr[:, :, off:512], ps[:, :, off:512],
                                 EXP, scale=SCALE)
            if diag:  # mask the 128-wide diagonal strip (both parities)
                mask_ns.tensor_mul(
                    pr[:, :, off:off + 128],
                    pr[:, :, off:off + 128],
                    mkt[:, None, :].broadcast_to([128, 2, 128]))
            st["ptiles"][kt] = (pr, off)
            st["i"] += 1
            return True

        def emit_pv_chunk(st, n=2):
            """Up to n PV key tiles per parity as same-bank matmul runs."""
            kts, j0 = st["kts"], st["j"]
            if j0 >= len(kts):
                return False
            if st["po"] is None:
                st["po"] = psO.tile([128, 2, 512], f32, tag="po", name="po")
            po = st["po"]
            j1 = min(j0 + n, len(kts))
            p = st["p"]
            for par in range(2):
                for j in range(j0, j1):
                    kt = kts[j]
                    pr, off = st["ptiles"][kt]
                    nc.tensor.matmul(
                        po[0:65, par, off:512],
                        vhat[:, kt, 2 * p + par, :],
                        pr[:, par, off:512],
                        start=(kt == 0), stop=(kt == kts[-1]))
            st["j"] = j1
            return True

        def emit_norm(st):
            """oT = po[0:64] * 1/po[64], both parities batched."""
            p, qc = st["p"], st["qc"]
            qsl = slice(qc * 512, (qc + 1) * 512)
            po = st["po"]
            ob = rp.tile([65, 2, 512], bf16, tag="ob", name="ob")
            if ob_eng == "act":
                nc.scalar.activation(ob[:, :, :], po[0:65, :, :], COPY)
            else:
                nc.vector.tensor_copy(ob[:, :, :], po[0:65, :, :])
            for par in range(2):
                bci = bcis[par]
                bci_ns.tensor_copy(bci[0:1, :], ob[64:65, par, :])
                bci_ns.tensor_copy(bci[32:33, :], ob[64:65, par, :])
                bc = rp.tile([64, 512], bf16, tag="bc", name="bc")
                nc.vector.stream_shuffle(bc[:, :], bci[:, :], [0] * 32)
                rden = rp.tile([64, 512], bf16, tag="rden", name="rden")
                with nc.allow_low_precision(
                        reason="bf16 softmax denominators"):
                    nc.vector.reciprocal(rden[:, :], bc[:, :])
                nc.vector.tensor_mul(oT[p][64 * par:64 * par + 64, qsl],
                                     ob[0:64, par, :], rden[:, :])

        def emit_c_mpair(qc, m0):
            """Output-projection m-tiles m0, m0+1 in one 2-bank PSUM tile."""
            ps = ws.tile([128, 2, 512], f32, tag="w", name="w")
            for i, m in ((0, m0), (1, m0 + 1)):
                for j in range(NPAIR):
                    nc.tensor.matmul(
                        ps[:, i, :], wpt[:, j, m * 128:(m + 1) * 128],
                        oT[j][:, qc * 512:(qc + 1) * 512],
                        start=(j == 0), stop=(j == NPAIR - 1))
            sb = ev.tile([128, 2, 512], f32, tag="sb", name="sb")
            nc.vector.tensor_copy(sb[:, :, :], ps[:, :, :])
            nc.sync.dma_start(
                out=yt[m0 * 128:(m0 + 2) * 128,
                       qc * 512:(qc + 1) * 512].rearrange(
                           "(t p) f -> p t f", p=128),
                in_=sb)

        def new_state(p, qc):
            kts = list(range(min(nkt, 4 * (qc + 1))))
            return {"p": p, "qc": qc, "kts": kts, "i": 0,
                    "j": 0, "ptiles": {}, "po": None}

        def retire(st):
            while emit_pv_chunk(st, n=4):
                pass
            emit_norm(st)
            if st["p"] == NPAIR - 1:
                pend_c.extend((st["qc"], m0) for m0 in range(0, nmt, 2))

        def drain_c(k):
            for _ in range(min(k, len(pend_c))):
                emit_c_mpair(*pend_c.pop(0))

        # ---------------- main pipeline ----------------
        # Filler discipline: PE is in-order, and each S step's matmuls
        # must wait for the previous step's exp (single-buffered [128,4,512]
        # S tile). So between S-mm batches PE needs ~2.5us of other queued
        # work: a PV chunk (n=4) plus one filler (V projection or C m-pair).
        for t in a_tasks(0):
            t()
        prev = None
        for idx, (p, qc) in enumerate(units):
            atasks = a_tasks(idx + 1)
            if atasks:
                atasks.pop(0)()          # q/k projection of next unit
            filler = atasks              # remaining V tasks, then C drains
            cur = new_state(p, qc)
            more_s = True
            while more_s:
                more_s = emit_s_step(cur)
                if prev is not None:
                    emit_pv_chunk(prev, n=2)
                    if prev["j"] >= len(prev["kts"]):
                        retire(prev)
                        prev = None
                if filler:
                    filler.pop(0)()
                elif pend_c and more_s:
                    emit_c_mpair(*pend_c.pop(0))
            for t in filler:
                t()
            if prev is not None:
                retire(prev)
            prev = cur
        if prev is not None:
            retire(prev)
        drain_c(len(pend_c))
    nc.compile()
    return nc


def _make_masks():
    import ml_dtypes
    k = np.arange(128)[:, None]
    q = np.arange(128)[None, :]
    return (q >= k).astype(ml_dtypes.bfloat16)


_NC_CACHE = {}


def _get_nc(tpc=T):
    if tpc not in _NC_CACHE:
        _NC_CACHE[tpc] = build_nc(tpc)
    return _NC_CACHE[tpc]


def make_in_maps(x, w_attn, w_proj):
    import ml_dtypes
    bf = ml_dtypes.bfloat16
    masks = _make_masks()
    in_maps = []
    for core in range(N_CORES):
        b, hh = core // 2, core % 2
        s = slice(hh * FS, (hh + 1) * FS)
        in_maps.append({
            "xt": np.ascontiguousarray(np.asarray(x[b]).T).astype(bf),
            "wq": np.ascontiguousarray(w_attn[:, s]).astype(bf),
            "wk": np.ascontiguousarray(w_attn[:, C:][:, s]).astype(bf),
            "wv": np.ascontiguousarray(w_attn[:, 2 * C:][:, s]).astype(bf),
            "wp": np.ascontiguousarray(w_proj[hh * FS:(hh + 1) * FS, :]).astype(bf),
            "mk": masks,
        })
    return in_maps


def kernel(x, w_attn, w_proj):
    nc = _get_nc(T)
    in_maps = make_in_maps(x, w_attn, w_proj)
    res = run_bass_kernel_spmd(nc, in_maps, list(range(N_CORES)))
    y = np.empty((B, T, C), np.float32)
    for b in range(B):
        yt = res.results[2 * b]["yt"] + res.results[2 * b + 1]["yt"]
        y[b] = yt.T
    return y
